# revision 33
# baseline (speedup 1.0000x reference)
"""GQA attention kernel for 8 TRN2 NeuronCores.

Problem: B=2, T=2048, C=4096, NH=32 q-heads, NKV=8 kv-heads, HD=128,
RoPE (theta=1e4), causal, f32 I/O.

Sharding: core = (batch b, kv-head-group g): b = core//4, g = core%4.
Each core owns batch b and kv heads {2g, 2g+1} (= q heads 8g..8g+7):
  - projects x[b] against its wq/wk/wv column slices (bf16 compute),
  - runs causal attention for its 8 q heads,
  - computes the partial o_proj x its wo row slice -> [T, C] f32.
Host sums the 4 partials per batch.

On-chip layout is feature-major ("X^T"): activations live as
[feature=partition, token=free] so every matmul contracts along
partitions. x is pre-transposed/bf16-cast on host; RoPE's rotate_half
is a 128x128 permutation matmul on the PE; softmax denominator comes
free from a ones-column appended to V.

Scheduling: all RoPE chunks (q and k heads) and V transposes run as
fillers interleaved into the projection stream, so their DVE chains
hide under projection matmuls and attention starts fully fed.
Attention zips head PAIRS: scores land two k-tiles per 2-bank PSUM
group (halving Act exp call count), exp is one batched Act call per
group, and PV runs two score-steps behind through a pending queue so
exp latency hides under the other head's matmuls. The j-pairs of the
PV accumulator share a PSUM bank, so po is memset once and accumulated
with start=False throughout (a second start=True in an open bank
corrupts the sibling accumulation group on HW). O-block transposes are
spread one-per-step through the FOLLOWING pair's stream (copy-back
alternating Act/DVE), leaving o_proj as a pure dense matmul phase with
8 psum banks.
"""

import sys

sys.path.insert(0, "/opt/trn_rl_repo")

import numpy as np
import ml_dtypes

import concourse.bass as bass
import concourse.bacc as bacc
import concourse.mybir as mybir
import concourse.tile as tile
from concourse.bass_utils import run_bass_kernel_spmd

BF16 = mybir.dt.bfloat16
F32 = mybir.dt.float32
AF = mybir.ActivationFunctionType
ALU = mybir.AluOpType

B, T, C = 2, 2048, 4096
NH, NKV, HD = 32, 8, 128
THETA = 10000.0
NCORES = 8

QH = 8          # q heads per core
KV = 2          # kv heads per core
QC = 4          # token chunks of 512
KT = 16         # k tiles of 128
TT = 16         # token tiles of 128
CCH = 32        # contraction chunks of 128 over C

_CACHE = {}


def _build_nc():
    nc = bacc.Bacc("TRN2", target_bir_lowering=False, debug=False,
                   enable_asserts=False, num_devices=NCORES)

    xT_d = nc.dram_tensor("xT", [C, T], BF16, kind="ExternalInput")
    wqkv_d = nc.dram_tensor("wqkv", [CCH, 2, 128, 768], BF16, kind="ExternalInput")
    wo_d = nc.dram_tensor("wo", [QH * HD, C], BF16, kind="ExternalInput")
    cos_d = nc.dram_tensor("cosT", [128, T], BF16, kind="ExternalInput")
    sin_d = nc.dram_tensor("sinT", [128, T], BF16, kind="ExternalInput")
    prot_d = nc.dram_tensor("protT", [128, 128], BF16, kind="ExternalInput")
    ident_d = nc.dram_tensor("ident", [128, 128], BF16, kind="ExternalInput")
    cmask_d = nc.dram_tensor("cmask", [128, 4, 512], F32, kind="ExternalInput")
    out_d = nc.dram_tensor("out", [T, C], F32, kind="ExternalOutput")

    with tile.TileContext(nc) as tc:
        with tc.tile_pool(name="persist", bufs=1) as pp:
            ident = pp.tile([128, 128], BF16)
            nc.sync.dma_start(ident, ident_d.ap())
            cosT = pp.tile([128, T], BF16)
            sinT = pp.tile([128, T], BF16)
            prot = pp.tile([128, 128], BF16)
            cmask = pp.tile([128, 4, 512], F32)
            # HAM warm-up: keep the PE busy while the first x^T block
            # DMAs in, so projections start at full clock.
            with tc.tile_pool(name="pwarm", bufs=2, space="PSUM") as pwp:
                for w in range(32):
                    wps = pwp.tile([128, 128], BF16, name=f"warm{w}", tag="warm")
                    nc.tensor.transpose(wps, ident, ident)

            QT = pp.tile([128, QH, T], BF16)
            KTt = pp.tile([128, KV, T], BF16)
            VT = pp.tile([128, KV, T], BF16)
            OT = pp.tile([128, QH, T], BF16)
            Vn = pp.tile([128, KV, KT, 132], BF16)
            nc.vector.memset(Vn[:, :, :, 128:129], 1.0)

            with tc.tile_pool(name="ropes", bufs=4) as rsp:

                def rope_chunk(src, rqc, psp, shape):
                    # q' = q*cos + (P_rot @ q)*sin on one 512-token chunk
                    rsl = slice(rqc * 512, (rqc + 1) * 512)
                    ps = psp.tile(shape, F32, tag="st")
                    nc.tensor.matmul(ps[:, 0:512], prot, src[:, rsl],
                                     start=True, stop=True)
                    rs = rsp.tile([128, 512], BF16)
                    nc.vector.tensor_tensor(rs, ps[:, 0:512], sinT[:, rsl],
                                            op=ALU.mult)
                    nc.vector.tensor_tensor(src[:, rsl], src[:, rsl],
                                            cosT[:, rsl], op=ALU.mult)
                    nc.vector.tensor_tensor(src[:, rsl], src[:, rsl], rs,
                                            op=ALU.add)

                def vtrans_tile(kv, kt, psp):
                    pt = psp.tile([128, 128], BF16, tag="st")
                    nc.tensor.transpose(
                        pt, VT[:, kv, kt * 128:(kt + 1) * 128], ident)
                    nc.vector.tensor_copy(Vn[:, kv, kt, 0:128], pt)

                # ---------------- projections: Q^T/K^T/V^T = W^T @ x^T ------
                with tc.tile_pool(name="xt", bufs=2) as xtp, \
                     tc.tile_pool(name="wt", bufs=5) as wtp, \
                     tc.tile_pool(name="pproj", bufs=6, space="PSUM") as ppj, \
                     tc.tile_pool(name="lead", bufs=2, space="PSUM") as ldp:

                    fillers = []
                    xview = xT_d.ap().rearrange("(c p) t -> p c t", p=128)
                    wview = wqkv_d.ap().rearrange("c g p o -> p c g o")
                    for qc in range(QC):
                        tsl = slice(qc * 512, (qc + 1) * 512)
                        xt = xtp.tile([128, CCH, 512], BF16)
                        for piece in range(4):
                            csl = slice(piece * 8, (piece + 1) * 8)
                            nc.scalar.dma_start(xt[:, csl, :], xview[:, csl, tsl])
                        if qc == 0:
                            # RoPE constants behind the first x^T block:
                            # landed long before the first rope filler
                            nc.scalar.dma_start(cosT, cos_d.ap())
                            nc.scalar.dma_start(sinT, sin_d.ap())
                            nc.scalar.dma_start(prot, prot_d.ap())
                        elif qc == 2:
                            # causal mask isn't needed until attention
                            nc.scalar.dma_start(cmask, cmask_d.ap())
                        for grp in range(2):
                            psums = [ppj.tile([128, 512], F32,
                                              name=f"pj{qc}_{grp}_{o}", tag="pj")
                                     for o in range(6)]
                            for c4 in range(CCH // 4):
                                wt = wtp.tile([128, 4, 768], BF16)
                                nc.sync.dma_start(
                                    wt, wview[:, 4 * c4:4 * c4 + 4, grp, :])
                                for cc in range(4):
                                    c = 4 * c4 + cc
                                    for o in range(6):
                                        nc.tensor.matmul(
                                            psums[o],
                                            wt[:, cc, o * 128:(o + 1) * 128],
                                            xt[:, c, :], start=(c == 0),
                                            stop=(c == CCH - 1))
                                    if fillers:
                                        fillers.pop(0)()
                            for o in range(6):
                                oi = grp * 6 + o
                                if oi < 8:
                                    dst = QT[:, oi, tsl]
                                elif oi < 10:
                                    dst = KTt[:, oi - 8, tsl]
                                else:
                                    dst = VT[:, oi - 10, tsl]
                                if o % 2 == 0:
                                    nc.scalar.copy(dst, psums[o])
                                else:
                                    nc.vector.tensor_copy(dst, psums[o])
                        # queue this chunk's lead work (runs in qc+1's
                        # filler slots; qc==3's remainder flushes below)
                        for kvh in range(KV):
                            fillers.append(lambda kvh=kvh, qc=qc: rope_chunk(
                                KTt[:, kvh, :], qc, ldp, [128, 512]))
                        for kvh in range(KV):
                            for kt in range(4 * qc, 4 * qc + 4):
                                fillers.append(
                                    lambda kvh=kvh, kt=kt: vtrans_tile(
                                        kvh, kt, ldp))
                        if qc < 3:
                            for h in range(QH):
                                fillers.append(lambda h=h, qc=qc: rope_chunk(
                                    QT[:, h, :], qc, ldp, [128, 512]))
                    # flush only the short-chain leftovers (K ropes, V
                    # transposes); qc3's Q ropes spread into pair0's
                    # score stream below
                    while fillers:
                        fillers.pop(0)()

                # wo load after the x^T/weight stream pools are gone, so it
                # overlaps attention without blowing SBUF
                wo_pool = tc.alloc_tile_pool(name="wop", bufs=1)
                wo_t = wo_pool.tile([128, QH, C], BF16)
                nc.sync.dma_start(
                    wo_t, wo_d.ap().rearrange("(h p) n -> p h n", p=128))

                # ---------------- attention (head-pair zipped) --------------
                # S^T[k,q] = K @ Q^T, two k-tiles per PSUM group;
                # P^T = exp(S^T + mask) batched per group on Act;
                # O^T accumulates P @ [V|1] in po (denominator = col 128).
                with tc.tile_pool(name="scorep", bufs=2, space="PSUM") as scp, \
                     tc.tile_pool(name="pop", bufs=4, space="PSUM") as pop, \
                     tc.tile_pool(name="ptp", bufs=8) as ptp, \
                     tc.tile_pool(name="rcp", bufs=8) as rcp:

                    po_tiles = {}
                    pending = []

                    def emit_scores(h, p, qc, kv):
                        tsl = slice(qc * 512, (qc + 1) * 512)
                        st = scp.tile([128, 1024], F32,
                                      name=f"st{h}_{qc}_{p}", tag="st")
                        for e in range(2):
                            kt = 2 * p + e
                            nc.tensor.matmul(
                                st[:, e * 512:(e + 1) * 512],
                                KTt[:, kv, kt * 128:(kt + 1) * 128],
                                QT[:, h, tsl], start=True, stop=True)
                        d0 = 2 * p - 4 * qc
                        for e in range(2):
                            d = 2 * p + e - 4 * qc
                            if 0 <= d < 4:
                                bsl = slice(e * 512 + d * 128,
                                            e * 512 + (d + 1) * 128)
                                nc.vector.tensor_tensor(
                                    st[:, bsl], st[:, bsl],
                                    cmask[:, d, d * 128:(d + 1) * 128],
                                    op=ALU.add)
                        pt = ptp.tile([128, 1024], BF16)
                        c0 = max(d0, 0) * 128
                        nc.scalar.activation(pt[:, c0:1024], st[:, c0:1024],
                                             AF.Exp)
                        return pt

                    def ensure_po(h, qc):
                        # Two j-accumulations share each po bank, so never
                        # use start=True (it corrupts the sibling group's
                        # partial sum): zero once, accumulate always.
                        for half in range(2):
                            key = (h, qc, half)
                            if key not in po_tiles:
                                pot = pop.tile([128, 2, 129], F32,
                                               name=f"po{h}_{qc}_{half}",
                                               tag="po")
                                nc.vector.memset(pot, 0.0)
                                po_tiles[key] = pot

                    def emit_pv(it):
                        h, p, qc, kv, pt = it
                        ensure_po(h, qc)
                        for e in range(2):
                            kt = 2 * p + e
                            d = kt - 4 * qc
                            for j in range(max(d, 0), 4):
                                qt = 4 * qc + j
                                nc.tensor.matmul(
                                    po_tiles[(h, qc, j // 2)][:, j % 2, :],
                                    pt[:, e * 512 + j * 128:
                                       e * 512 + (j + 1) * 128],
                                    Vn[:, kv, kt, 0:129],
                                    start=False, stop=(kt == qt),
                                    skip_group_check=True)

                    def emit_norm(h, qc):
                        # 1/denominator (po col 128); store normalized O
                        # token-major into OT (transposed at pair boundary)
                        for half in range(2):
                            pot = po_tiles.pop((h, qc, half))
                            rc = rcp.tile([128, 2], F32,
                                          name=f"rc{h}_{qc}_{half}")
                            nc.vector.reciprocal(rc, pot[:, :, 128])
                            for jj in range(2):
                                j = half * 2 + jj
                                qt = 4 * qc + j
                                nc.vector.tensor_scalar_mul(
                                    OT[:, h, qt * 128:(qt + 1) * 128],
                                    pot[:, jj, 0:128], rc[:, jj:jj + 1])

                    def pop_one():
                        it = pending.pop(0)
                        emit_pv(it)
                        if it[1] == 2 * it[2] + 1:  # last k-pair of its qc
                            emit_norm(it[0], it[2])

                    def ot_tile(h, qt, eng):
                        # in-place transpose one O block [tok,hd] -> [hd,tok]
                        osl = slice(qt * 128, (qt + 1) * 128)
                        ptr = pop.tile([128, 128], BF16,
                                       name=f"otr{h}_{qt}", tag="po")
                        nc.tensor.transpose(ptr, OT[:, h, osl], ident)
                        if eng == 0:
                            nc.scalar.copy(OT[:, h, osl], ptr)
                        else:
                            nc.vector.tensor_copy(OT[:, h, osl], ptr)

                    ot_todo = []
                    rope3_todo = list(range(QH))  # qc3 Q ropes, h0/h1 first
                    for idx, ha in enumerate((0, 2, 4, 6)):
                        hb, kv = ha + 1, ha // 4
                        step = 0
                        for qc in range(QC):
                            for p in range(2 * qc + 2):
                                for h in (ha, hb):
                                    pending.append(
                                        (h, p, qc, kv,
                                         emit_scores(h, p, qc, kv)))
                                    while len(pending) > 5:
                                        pop_one()
                                    # spread previous pair's O transposes
                                    # through this pair's LATER steps, where
                                    # qc transitions leave the PE short of
                                    # queued filler work
                                    if step >= 8 and ot_todo:
                                        h2, qt2 = ot_todo.pop(0)
                                        ot_tile(h2, qt2, step % 2)
                                    if step % 4 == 2 and rope3_todo:
                                        rope_chunk(QT[:, rope3_todo.pop(0), :],
                                                   3, scp, [128, 1024])
                                    step += 1
                        while pending:
                            pop_one()
                        while ot_todo:  # few leftovers (short pairs)
                            h2, qt2 = ot_todo.pop(0)
                            ot_tile(h2, qt2, len(ot_todo) % 2)
                        # interleave (ha,qt),(hb,qt) so o_proj's qt order
                        # unblocks earliest
                        ot_todo = [(h, qt) for qt in range(TT)
                                   for h in (ha, hb)]
                    # last pair's transposes: small post-drain blob,
                    # qt-ascending so o_proj tt=0 unblocks first
                    for i, (h2, qt2) in enumerate(ot_todo):
                        ot_tile(h2, qt2, i % 2)
                    ot_todo = []

            # ---------------- o_proj partial: O @ wo_slice ----------------
            with tc.tile_pool(name="pout", bufs=8, space="PSUM") as outp, \
                 tc.tile_pool(name="ostg", bufs=6) as stgp:
                for tt in range(TT):
                    psl = slice(tt * 128, (tt + 1) * 128)
                    for n in range(8):
                        nsl = slice(n * 512, (n + 1) * 512)
                        ps = outp.tile([128, 512], F32)
                        for h in range(QH):
                            nc.tensor.matmul(ps, OT[:, h, psl],
                                             wo_t[:, h, nsl],
                                             start=(h == 0), stop=(h == QH - 1))
                        stg = stgp.tile([128, 512], F32)
                        if n % 2 == 0:
                            nc.scalar.copy(stg, ps)
                        else:
                            nc.vector.tensor_copy(stg, ps)
                        nc.sync.dma_start(out_d.ap()[psl, nsl], stg)

            wo_pool.release()

    nc.compile()
    return nc


def _host_prep(x, wq, wk, wv, wo):
    bf = ml_dtypes.bfloat16
    scale = HD ** -0.5

    # RoPE tables, feature-major [128, T]
    inv_freq = 1.0 / (THETA ** (np.arange(0, HD, 2, dtype=np.float32) / HD))
    t = np.arange(T, dtype=np.float32)
    freqs = np.outer(t, inv_freq)                      # [T, 64]
    emb = np.concatenate([freqs, freqs], -1)           # [T, 128]
    cosT = np.ascontiguousarray(np.cos(emb).T).astype(bf)
    sinT = np.ascontiguousarray(np.sin(emb).T).astype(bf)

    # rotate_half as a permutation matrix, pre-transposed for lhsT:
    # rot = P_rot @ q with P_rot[i, i+64] = -1 (i<64), P_rot[i, i-64] = +1.
    protT = np.zeros((128, 128), np.float32)
    for i in range(64):
        protT[i + 64, i] = -1.0
        protT[i, i + 64] = 1.0
    protT = protT.astype(bf)

    ident = np.eye(128, dtype=np.float32).astype(bf)

    # additive causal masks for the 4 diagonal [128k, 512q] tiles
    # valid iff q_local >= d*128 + k_local
    kl = np.arange(128)[:, None]
    ql = np.arange(512)[None, :]
    cmask = np.stack(
        [np.where(ql >= d * 128 + kl, 0.0, -1e9).astype(np.float32)
         for d in range(4)], axis=1)                   # [128, 4, 512]
    cmask = np.ascontiguousarray(cmask)

    xT = []
    for b in range(B):
        xT.append(np.ascontiguousarray(x[b].astype(bf).T))

    wqkv, wob = [], []
    for g in range(4):
        q_s = (wq[:, g * 1024:(g + 1) * 1024] * scale).astype(bf)
        k_s = wk[:, g * 256:(g + 1) * 256].astype(bf)
        v_s = wv[:, g * 256:(g + 1) * 256].astype(bf)
        wall = np.concatenate([q_s, k_s, v_s], axis=1)       # [C, 1536]
        wall = wall.reshape(CCH, 128, 2, 768).transpose(0, 2, 1, 3)
        wqkv.append(np.ascontiguousarray(wall))              # [32, 2, 128, 768]
        wob.append(np.ascontiguousarray(
            wo[g * 1024:(g + 1) * 1024, :].astype(bf)))      # [1024, C]

    in_maps = []
    for core in range(NCORES):
        b, g = core // 4, core % 4
        in_maps.append({
            "xT": xT[b], "wqkv": wqkv[g], "wo": wob[g],
            "cosT": cosT, "sinT": sinT, "protT": protT,
            "ident": ident, "cmask": cmask,
        })
    return in_maps


def kernel(x, wq, wk, wv, wo, _trace=False, _tmpdir=None):
    if "nc" not in _CACHE:
        _CACHE["nc"] = _build_nc()
    nc = _CACHE["nc"]

    in_maps = _host_prep(x, wq, wk, wv, wo)
    res = run_bass_kernel_spmd(nc, in_maps, core_ids=list(range(NCORES)),
                               trace=_trace, tmpdir=_tmpdir)
    _CACHE["last_results"] = res

    out = np.zeros((B, T, C), np.float32)
    for core in range(NCORES):
        out[core // 4] += res.results[core]["out"]
    return out
